# revision 10
# baseline (speedup 1.0000x reference)
"""Trainium2 Bass kernel for nn_Attention_84327387890534.

Multi-head attention with 1D relative position bias:
  x = x + noise * noise_strength
  qkv = x @ w_qkv -> q,k,v per head
  attn = softmax(q k^T * hd^-0.5 + rel_bias[i-j])
  out = (attn @ v) @ w_proj + b_proj

Sharding: data-parallel over batch B=8, one batch per NeuronCore.

Per-core design (all matmuls fp16 operands, fp32 PSUM accumulation):
  - x loaded fp32, noise-add fused into the fp16 cast (ACT per-partition
    bias), then PE-transposed to xT [c, n].
  - q,k computed transposed (qT/kT = [head*hd + d, n]) so scores need no
    per-head transposes; v computed natural [n, c] and packed per head as
    [v_h | ones] so the attn@v matmul also emits softmax row-sums for free
    (out rows 0..63 = unnormalized out^T, rows 64..127 = replicated rowsum).
  - scores computed transposed: S^T[j, p] tiles, softmax along free dim.
    Bias applied multiplicatively: exp(S + bias) = exp(S) * exp(bias); the
    exp(bias) Toeplitz tiles are generated by sliding-window DMAs (negative
    outer stride) from a per-head 2047-entry exp'd table staged in DRAM.
  - attn^T tiles feed the attn@v matmul directly (no transposes anywhere
    in the attention path); output is attnout^T [cin, n], which is exactly
    the lhsT layout the projection matmul needs.
"""

import sys

import numpy as np
from contextlib import ExitStack

try:
    import concourse.bass as bass
except ImportError:  # pragma: no cover
    sys.path.insert(0, "/opt/trn_rl_repo")
    import concourse.bass as bass

import concourse.tile as tile
from concourse import mybir
from concourse.bass_utils import run_bass_kernel_spmd
from concourse.masks import make_identity

F32 = mybir.dt.float32
F16 = mybir.dt.float16

# --- workaround: this walrus build rejects >1 sync-wait command on a single
# TPB_CTRL (Drain) instruction; TileContext's tail drain attaches every
# outstanding semaphore wait to one drain. Split the waits across extra
# drain instructions (2 per instruction) before the all-engine barrier.
_MAX_WAITS_PER_CTRL = 1


def _split_drain_and_barrier(self, tick_clock, wait_clock):
    import bass_rust
    from concourse.vector_clock import ScopedClock

    nc = self.nc
    drain_inst = nc.sync.drain()
    wait_clock.add_sem_waits(
        drain_inst.ins, ScopedClock({None: tick_clock.global_clock})
    )
    mi = drain_inst.ins
    si = mi.sync_info
    if si is not None and si.on_wait and len(si.on_wait) > _MAX_WAITS_PER_CTRL:
        waits = list(si.on_wait)
        mi.sync_info = bass_rust.SyncInfo(
            on_wait=waits[:_MAX_WAITS_PER_CTRL], on_update=list(si.on_update)
        )
        for i in range(_MAX_WAITS_PER_CTRL, len(waits), _MAX_WAITS_PER_CTRL):
            extra = nc.sync.drain()
            extra.ins.sync_info = bass_rust.SyncInfo(
                on_wait=waits[i:i + _MAX_WAITS_PER_CTRL], on_update=[]
            )

    nc.all_engine_barrier()
    assert self.sems is not None
    popped = nc._tile_sem_poison_stack.pop()
    assert popped is self._sem_poison
    nc.clear_and_free_semaphores(list(self.sems.allocated().values()))
    nc.all_engine_barrier()


tile.TileContext._drain_and_barrier = _split_drain_and_barrier


def _split_multi_waits(nc, max_waits=_MAX_WAITS_PER_CTRL):
    """Walrus here emits at most one sync-wait command per TPB instruction.
    Move excess semaphore waits onto same-engine NoOps inserted just before
    the over-subscribed instruction (identical semantics: engine streams
    are sequential, so the waits still all complete first)."""
    import bass_rust

    for fn in nc.m.functions:
        for bb in fn.blocks:
            out = []
            changed = False
            for inst in bb.instructions:
                si = inst.sync_info
                if si is not None and si.on_wait and len(si.on_wait) > max_waits:
                    waits = list(si.on_wait)
                    extras, keep = waits[:-max_waits], waits[-max_waits:]
                    for i in range(0, len(extras), max_waits):
                        nop = mybir.InstNoOp(
                            name=nc.get_next_instruction_name(), ins=[], outs=[]
                        )
                        nop.engine = inst.engine
                        nop.sync_info = bass_rust.SyncInfo(
                            on_wait=extras[i:i + max_waits], on_update=[]
                        )
                        nc.register_instruction(nop, overwrite=True)
                        out.append(nop)
                    inst.sync_info = bass_rust.SyncInfo(
                        on_wait=keep, on_update=list(si.on_update)
                    )
                    changed = True
                out.append(inst)
            if changed:
                bb.instructions = out
    return nc

# Problem dimensions (hardcoded per harness contract).
B = 8
N = 1024
C = 1024
H = 16
HD = 64
NCORES = 8


def build(n=N, c=C, h=H, hd=HD):
    """Build the single-core SPMD Bass program."""
    assert hd == 64 and c == h * hd and n % 128 == 0 and c % 128 == 0
    ws = n
    tbl_len = 2 * ws - 1
    nb, cb = n // 128, c // 128
    qk_tiles = 2 * cb
    scale = float(hd) ** -0.5
    n512 = [(j0, min(512, n - j0)) for j0 in range(0, n, 512)]
    c512 = [(j0, min(512, c - j0)) for j0 in range(0, c, 512)]

    nc = bass.Bass(trn_type="TRN2")
    x_d = nc.declare_dram_parameter("x", [n, c], F32, isOutput=False)
    nz_d = nc.declare_dram_parameter("noise", [n, 1], F32, isOutput=False)
    ns_d = nc.declare_dram_parameter("nstr", [1, 1], F32, isOutput=False)
    wqk_d = nc.declare_dram_parameter("wqk", [cb, qk_tiles, 128, 128], F16, isOutput=False)
    wv_d = nc.declare_dram_parameter("wv", [c, c], F16, isOutput=False)
    wp_d = nc.declare_dram_parameter("wproj", [c, c], F16, isOutput=False)
    bp_d = nc.declare_dram_parameter("bproj", [c], F32, isOutput=False)
    tb_d = nc.declare_dram_parameter("tbl", [tbl_len, h], F32, isOutput=False)
    out_d = nc.declare_dram_parameter("out", [n, c], F32, isOutput=True)

    with ExitStack() as ctx:
        tc = ctx.enter_context(tile.TileContext(nc))
        const = ctx.enter_context(tc.tile_pool(name="const", bufs=1))
        dramp = ctx.enter_context(tc.tile_pool(name="dram", bufs=1, space="DRAM"))

        ident = const.tile([128, 128], F16, tag="ident")
        make_identity(nc, ident)

        # exp(bias) table, transposed to [h, tbl_len] fp16, staged to DRAM.
        tblT = const.tile([h, tbl_len], F32, tag="tblT")
        nc.sync.dma_start(
            out=tblT,
            in_=bass.AP(tensor=tb_d[:].tensor, offset=0, ap=[[1, h], [h, tbl_len]]),
        )
        ebt_sb = const.tile([h, tbl_len], F16, tag="ebt_sb")
        nc.scalar.activation(ebt_sb, tblT, mybir.ActivationFunctionType.Exp)
        ebt = dramp.tile([h, tbl_len], F16)
        nc.sync.dma_start(out=ebt[:], in_=ebt_sb[:])
        ebt_ap = ebt[:]

        # noise * noise_strength: column a holds the bias for n-block a.
        nstr = const.tile([128, 1], F32, tag="nstr")
        nc.sync.dma_start(
            out=nstr,
            in_=bass.AP(tensor=ns_d[:].tensor, offset=0, ap=[[0, 128], [1, 1]]),
        )
        noise_sb = const.tile([128, nb], F32, tag="noise")
        nc.sync.dma_start(
            out=noise_sb,
            in_=bass.AP(tensor=nz_d[:].tensor, offset=0, ap=[[1, 128], [128, nb]]),
        )
        noise_sc = const.tile([128, nb], F32, tag="noise_sc")
        nc.vector.tensor_scalar_mul(noise_sc, noise_sb, nstr)

        # b_proj broadcast to all partitions.
        bp_rep = const.tile([128, c], F32, tag="bp")
        nc.sync.dma_start(
            out=bp_rep,
            in_=bass.AP(tensor=bp_d[:].tensor, offset=0, ap=[[0, 128], [1, c]]),
        )

        # Persistent activations. Key order (j) is REVERSED within each
        # 128-chunk throughout the attention path (kT tiles, xTr -> vjones)
        # so the exp(bias) Toeplitz tiles become all-positive-stride Hankel
        # reads (walrus rejects negative outer strides in DMAs).
        acts = ctx.enter_context(tc.tile_pool(name="acts", bufs=1))
        xT = acts.tile([128, cb, n], F16, tag="xT")
        xTr = acts.tile([128, cb, n], F16, tag="xTr")
        qkT = [acts.tile([128, n], F16, tag=f"qkT{i}", name=f"qkT{i}") for i in range(qk_tiles)]
        vjones = [acts.tile([128, h, 2 * hd], F16, tag=f"vj{i}", name=f"vj{i}") for i in range(nb)]
        aoT = [acts.tile([128, n], F16, tag=f"aoT{i}", name=f"aoT{i}") for i in range(cb)]

        def rev_inner(ap_src, outer_pairs):
            """View with the innermost 128-wide free dim reversed."""
            return bass.AP(
                tensor=ap_src.tensor,
                offset=ap_src.offset + 127,
                ap=[ap_src.ap[0]] + outer_pairs + [[-1, 128]],
            )

        # ---- phase 1: x + noise -> fp16 -> xT via PE transpose
        with tc.tile_pool(name="ph1", bufs=3) as p1, \
             tc.tile_pool(name="ph1p", bufs=2, space="PSUM") as p1p:
            for a in range(nb):
                x32 = p1.tile([128, c], F32, tag="x32")
                nc.sync.dma_start(out=x32, in_=x_d[a * 128:(a + 1) * 128, :])
                xp = p1.tile([128, c], F16, tag="xp")
                nc.scalar.activation(
                    xp, x32, mybir.ActivationFunctionType.Identity,
                    bias=noise_sc[:, a:a + 1], scale=1.0,
                )
                ptp = p1p.tile([128, cb, 128], F16)
                for cc in range(cb):
                    nc.tensor.transpose(
                        out=ptp[:, cc, :], in_=xp[:, cc * 128:(cc + 1) * 128],
                        identity=ident,
                    )
                nc.scalar.copy(xT[:, :, a * 128:(a + 1) * 128], ptp[:])
                nc.scalar.copy(
                    xTr[:, :, a * 128:(a + 1) * 128],
                    rev_inner(ptp[:], [[128, cb]]),
                )

        # ---- phase 2: qT / kT = w_qk_block.T @ xT
        with tc.tile_pool(name="ph2w", bufs=6) as p2w, \
             tc.tile_pool(name="ph2p", bufs=2, space="PSUM") as p2p:
            for colb in range(qk_tiles):
                ps = p2p.tile([128, n], F32)
                for cc in range(cb):
                    wblk = p2w.tile([128, 128], F16)
                    nc.sync.dma_start(out=wblk, in_=wqk_d[cc, colb, :, :])
                    for j0, jl in n512:
                        nc.tensor.matmul(
                            ps[:, j0:j0 + jl], wblk, xT[:, cc, j0:j0 + jl],
                            start=(cc == 0), stop=(cc == cb - 1),
                        )
                if colb < cb:
                    nc.scalar.copy(qkT[colb], ps)
                else:
                    # k tiles: reverse key order within each 128-chunk
                    nc.scalar.copy(qkT[colb], rev_inner(ps[:], [[128, nb]]))

        # ---- phase 3: v = xT_block.T @ w_v, packed as [v_h | ones]
        with tc.tile_pool(name="ph3w", bufs=1) as p3w, \
             tc.tile_pool(name="ph3p", bufs=2, space="PSUM") as p3p:
            wv_sb = [p3w.tile([128, c], F16, tag=f"wv{cc}", name=f"wv{cc}") for cc in range(cb)]
            for cc in range(cb):
                nc.sync.dma_start(out=wv_sb[cc], in_=wv_d[cc * 128:(cc + 1) * 128, :])
            for a in range(nb):
                ps = p3p.tile([128, c], F32)
                for cc in range(cb):
                    for j0, jl in c512:
                        nc.tensor.matmul(
                            ps[:, j0:j0 + jl],
                            xTr[:, cc, a * 128:(a + 1) * 128],
                            wv_sb[cc][:, j0:j0 + jl],
                            start=(cc == 0), stop=(cc == cb - 1),
                        )
                nc.scalar.copy(
                    vjones[a][:, :, 0:hd],
                    ps.rearrange("p (hh d) -> p hh d", hh=h),
                )
                nc.vector.memset(vjones[a][:, :, hd:2 * hd], 1.0)

        # ---- phase 4: attention per (head, key-block)
        with tc.tile_pool(name="ph4e", bufs=4) as p4e, \
             tc.tile_pool(name="ph4x", bufs=3) as p4x, \
             tc.tile_pool(name="ph4a", bufs=3) as p4a, \
             tc.tile_pool(name="ph4f", bufs=2) as p4f, \
             tc.tile_pool(name="ph4ps", bufs=2, space="PSUM") as p4ps, \
             tc.tile_pool(name="ph4po", bufs=2, space="PSUM") as p4po:
            for hh in range(h):
                qt_i, qt_o = (hh * hd) // 128, (hh * hd) % 128
                qT_ap = qkT[qt_i][qt_o:qt_o + hd, :]
                kT_ap = qkT[cb + qt_i][qt_o:qt_o + hd, :]
                po = p4po.tile([128, n], F32)
                for jb in range(nb):
                    ps = p4ps.tile([128, n], F32)
                    for j0, jl in n512:
                        nc.tensor.matmul(
                            ps[:, j0:j0 + jl],
                            kT_ap[:, jb * 128:(jb + 1) * 128],
                            qT_ap[:, j0:j0 + jl],
                            start=True, stop=True,
                        )
                    et = p4e.tile([128, n], F16)
                    # row r holds key j = jb*128 + (127 - r); bias value is
                    # ebt[h, p - j + ws - 1] = ebt[h, (ws-128-128*jb) + r + p]
                    a0 = ws - 128 - 128 * jb
                    nc.sync.dma_start(
                        out=et,
                        in_=bass.AP(
                            tensor=ebt_ap.tensor,
                            offset=ebt_ap.offset + hh * tbl_len + a0,
                            ap=[[1, 128], [1, n]],
                        ),
                    )
                    ex = p4x.tile([128, n], F16)
                    nc.scalar.activation(
                        ex, ps, mybir.ActivationFunctionType.Exp, scale=scale,
                    )
                    at = p4a.tile([128, n], F16)
                    nc.vector.tensor_tensor(at, ex, et, op=mybir.AluOpType.mult)
                    for j0, jl in n512:
                        nc.tensor.matmul(
                            po[:, j0:j0 + jl],
                            vjones[jb][:, hh, :],
                            at[:, j0:j0 + jl],
                            start=(jb == 0), stop=(jb == nb - 1),
                        )
                rc = p4f.tile([64, n], F32)
                nc.vector.reciprocal(rc, po[64:128, :])
                nc.vector.tensor_tensor(
                    aoT[qt_i][qt_o:qt_o + hd, :], po[0:hd, :], rc,
                    op=mybir.AluOpType.mult,
                )

        # ---- phase 5: out = attnout^T.T @ w_proj + b_proj
        with tc.tile_pool(name="ph5w", bufs=1) as p5w, \
             tc.tile_pool(name="ph5o", bufs=3) as p5o, \
             tc.tile_pool(name="ph5p", bufs=2, space="PSUM") as p5p:
            wp_sb = [p5w.tile([128, c], F16, tag=f"wp{cc}", name=f"wp{cc}") for cc in range(cb)]
            for cc in range(cb):
                nc.sync.dma_start(out=wp_sb[cc], in_=wp_d[cc * 128:(cc + 1) * 128, :])
            for a in range(nb):
                ps = p5p.tile([128, c], F32)
                for cc in range(cb):
                    for j0, jl in c512:
                        nc.tensor.matmul(
                            ps[:, j0:j0 + jl],
                            aoT[cc][:, a * 128:(a + 1) * 128],
                            wp_sb[cc][:, j0:j0 + jl],
                            start=(cc == 0), stop=(cc == cb - 1),
                        )
                ob = p5o.tile([128, c], F32)
                nc.vector.tensor_tensor(ob, ps, bp_rep[:, 0:c], op=mybir.AluOpType.add)
                nc.sync.dma_start(out=out_d[a * 128:(a + 1) * 128, :], in_=ob)

    return _split_multi_waits(nc)


def prep_core_inputs(x2d, noise2d, w_qkv, w_proj, b_proj, tbl, nstr, c=C):
    """Host-side input prep for one core: fp16 weight casts + blocking."""
    cb = c // 128
    qk_tiles = 2 * cb
    wqk = np.ascontiguousarray(
        w_qkv[:, : 2 * c].astype(np.float16)
        .reshape(cb, 128, qk_tiles, 128)
        .transpose(0, 2, 1, 3)
    )
    return dict(
        x=np.ascontiguousarray(x2d, dtype=np.float32),
        noise=np.ascontiguousarray(noise2d, dtype=np.float32),
        nstr=np.asarray(nstr, dtype=np.float32).reshape(1, 1),
        wqk=wqk,
        wv=np.ascontiguousarray(w_qkv[:, 2 * c:].astype(np.float16)),
        wproj=np.ascontiguousarray(w_proj.astype(np.float16)),
        bproj=np.ascontiguousarray(b_proj, dtype=np.float32),
        tbl=np.ascontiguousarray(tbl, dtype=np.float32),
    )


_NC_CACHE = {}


def get_nc():
    if "nc" not in _NC_CACHE:
        _NC_CACHE["nc"] = build()
    return _NC_CACHE["nc"]


def kernel(**inputs):
    x = np.asarray(inputs["x"], dtype=np.float32)
    noise = np.asarray(inputs["noise"], dtype=np.float32)
    w_qkv = np.asarray(inputs["w_qkv"], dtype=np.float32)
    w_proj = np.asarray(inputs["w_proj"], dtype=np.float32)
    b_proj = np.asarray(inputs["b_proj"], dtype=np.float32)
    tbl = np.asarray(inputs["rel_bias_table"], dtype=np.float32)
    nstr = np.asarray(inputs["noise_strength"], dtype=np.float32)

    shared = None
    in_maps = []
    for i in range(B):
        m = prep_core_inputs(x[i], noise[i], w_qkv, w_proj, b_proj, tbl, nstr)
        if shared is None:
            shared = {k: v for k, v in m.items() if k not in ("x", "noise")}
        else:
            for k in shared:
                m[k] = shared[k]
        in_maps.append(m)

    res = run_bass_kernel_spmd(get_nc(), in_maps, list(range(NCORES))).results
    return np.stack([res[i]["out"] for i in range(B)], axis=0).astype(np.float32)


if __name__ == "__main__":
    nc = build()
    print("build ok")


# revision 13
# speedup vs baseline: 1.1463x; 1.1463x over previous
"""Trainium2 Bass kernel for nn_Attention_84327387890534.

Multi-head attention with 1D relative position bias:
  x = x + noise * noise_strength
  qkv = x @ w_qkv -> q,k,v per head
  attn = softmax(q k^T * hd^-0.5 + rel_bias[i-j])
  out = (attn @ v) @ w_proj + b_proj

Sharding: data-parallel over batch B=8, one batch per NeuronCore.

Per-core design (all matmuls fp16 operands, fp32 PSUM accumulation):
  - x loaded fp32, noise-add fused into the fp16 cast (ACT per-partition
    bias), then PE-transposed to xT [c, n].
  - q,k computed transposed (qT/kT = [head*hd + d, n]) so scores need no
    per-head transposes; v computed natural [n, c] and packed per head as
    [v_h | ones] so the attn@v matmul also emits softmax row-sums for free
    (out rows 0..63 = unnormalized out^T, rows 64..127 = replicated rowsum).
  - scores computed transposed: S^T[j, p] tiles, softmax along free dim.
    Bias applied multiplicatively: exp(S + bias) = exp(S) * exp(bias); the
    exp(bias) Toeplitz tiles are generated by sliding-window DMAs (negative
    outer stride) from a per-head 2047-entry exp'd table staged in DRAM.
  - attn^T tiles feed the attn@v matmul directly (no transposes anywhere
    in the attention path); output is attnout^T [cin, n], which is exactly
    the lhsT layout the projection matmul needs.
"""

import sys

import numpy as np
from contextlib import ExitStack

try:
    import concourse.bass as bass
except ImportError:  # pragma: no cover
    sys.path.insert(0, "/opt/trn_rl_repo")
    import concourse.bass as bass

import concourse.tile as tile
from concourse import mybir
from concourse.bass_utils import run_bass_kernel_spmd
from concourse.masks import make_identity

F32 = mybir.dt.float32
F16 = mybir.dt.float16

# --- workaround: this walrus build rejects >1 sync-wait command on a single
# TPB_CTRL (Drain) instruction; TileContext's tail drain attaches every
# outstanding semaphore wait to one drain. Split the waits across extra
# drain instructions (2 per instruction) before the all-engine barrier.
_MAX_WAITS_PER_CTRL = 1


def _split_drain_and_barrier(self, tick_clock, wait_clock):
    import bass_rust
    from concourse.vector_clock import ScopedClock

    nc = self.nc
    drain_inst = nc.sync.drain()
    wait_clock.add_sem_waits(
        drain_inst.ins, ScopedClock({None: tick_clock.global_clock})
    )
    mi = drain_inst.ins
    si = mi.sync_info
    if si is not None and si.on_wait and len(si.on_wait) > _MAX_WAITS_PER_CTRL:
        waits = list(si.on_wait)
        mi.sync_info = bass_rust.SyncInfo(
            on_wait=waits[:_MAX_WAITS_PER_CTRL], on_update=list(si.on_update)
        )
        for i in range(_MAX_WAITS_PER_CTRL, len(waits), _MAX_WAITS_PER_CTRL):
            extra = nc.sync.drain()
            extra.ins.sync_info = bass_rust.SyncInfo(
                on_wait=waits[i:i + _MAX_WAITS_PER_CTRL], on_update=[]
            )

    nc.all_engine_barrier()
    assert self.sems is not None
    popped = nc._tile_sem_poison_stack.pop()
    assert popped is self._sem_poison
    nc.clear_and_free_semaphores(list(self.sems.allocated().values()))
    nc.all_engine_barrier()


tile.TileContext._drain_and_barrier = _split_drain_and_barrier


def _split_multi_waits(nc, max_waits=_MAX_WAITS_PER_CTRL):
    """Walrus here emits at most one sync-wait command per TPB instruction.
    Move excess semaphore waits onto same-engine NoOps inserted just before
    the over-subscribed instruction (identical semantics: engine streams
    are sequential, so the waits still all complete first)."""
    import bass_rust

    for fn in nc.m.functions:
        for bb in fn.blocks:
            out = []
            changed = False
            for inst in bb.instructions:
                si = inst.sync_info
                if si is not None and si.on_wait and len(si.on_wait) > max_waits:
                    waits = list(si.on_wait)
                    extras, keep = waits[:-max_waits], waits[-max_waits:]
                    for i in range(0, len(extras), max_waits):
                        nop = mybir.InstNoOp(
                            name=nc.get_next_instruction_name(), ins=[], outs=[]
                        )
                        nop.engine = inst.engine
                        nop.sync_info = bass_rust.SyncInfo(
                            on_wait=extras[i:i + max_waits], on_update=[]
                        )
                        nc.register_instruction(nop, overwrite=True)
                        out.append(nop)
                    inst.sync_info = bass_rust.SyncInfo(
                        on_wait=keep, on_update=list(si.on_update)
                    )
                    changed = True
                out.append(inst)
            if changed:
                bb.instructions = out
    return nc

# Problem dimensions (hardcoded per harness contract).
B = 8
N = 1024
C = 1024
H = 16
HD = 64
NCORES = 8


def build(n=N, c=C, h=H, hd=HD):
    """Build the single-core SPMD Bass program."""
    assert hd == 64 and c == h * hd and n % 128 == 0 and c % 128 == 0
    ws = n
    tbl_len = 2 * ws - 1
    nb, cb = n // 128, c // 128
    qk_tiles = 2 * cb
    scale = float(hd) ** -0.5
    n512 = [(j0, min(512, n - j0)) for j0 in range(0, n, 512)]
    c512 = [(j0, min(512, c - j0)) for j0 in range(0, c, 512)]

    nc = bass.Bass(trn_type="TRN2")
    x_d = nc.declare_dram_parameter("x", [n, c], F32, isOutput=False)
    nz_d = nc.declare_dram_parameter("noise", [n, 1], F32, isOutput=False)
    ns_d = nc.declare_dram_parameter("nstr", [1, 1], F32, isOutput=False)
    wqk_d = nc.declare_dram_parameter("wqk", [cb, qk_tiles, 128, 128], F16, isOutput=False)
    wv_d = nc.declare_dram_parameter("wv", [c, c], F16, isOutput=False)
    wp_d = nc.declare_dram_parameter("wproj", [c, c], F16, isOutput=False)
    bp_d = nc.declare_dram_parameter("bproj", [c], F32, isOutput=False)
    tb_d = nc.declare_dram_parameter("tbl", [tbl_len, h], F32, isOutput=False)
    out_d = nc.declare_dram_parameter("out", [n, c], F32, isOutput=True)

    with ExitStack() as ctx:
        tc = ctx.enter_context(tile.TileContext(nc))
        const = ctx.enter_context(tc.tile_pool(name="const", bufs=1))
        dramp = ctx.enter_context(tc.tile_pool(name="dram", bufs=1, space="DRAM"))

        ident = const.tile([128, 128], F16, tag="ident")
        make_identity(nc, ident)

        # exp(bias) table, transposed to [h, tbl_len] fp16, staged to DRAM.
        tblT = const.tile([h, tbl_len], F32, tag="tblT")
        nc.sync.dma_start(
            out=tblT,
            in_=bass.AP(tensor=tb_d[:].tensor, offset=0, ap=[[1, h], [h, tbl_len]]),
        )
        ebt_sb = const.tile([h, tbl_len], F16, tag="ebt_sb")
        nc.scalar.activation(ebt_sb, tblT, mybir.ActivationFunctionType.Exp)
        ebt = dramp.tile([h, tbl_len], F16)
        nc.sync.dma_start(out=ebt[:], in_=ebt_sb[:])
        ebt_ap = ebt[:]

        # noise * noise_strength: column a holds the bias for n-block a.
        nstr = const.tile([128, 1], F32, tag="nstr")
        nc.sync.dma_start(
            out=nstr,
            in_=bass.AP(tensor=ns_d[:].tensor, offset=0, ap=[[0, 128], [1, 1]]),
        )
        noise_sb = const.tile([128, nb], F32, tag="noise")
        nc.sync.dma_start(
            out=noise_sb,
            in_=bass.AP(tensor=nz_d[:].tensor, offset=0, ap=[[1, 128], [128, nb]]),
        )
        noise_sc = const.tile([128, nb], F32, tag="noise_sc")
        nc.vector.tensor_scalar_mul(noise_sc, noise_sb, nstr)

        # b_proj broadcast to all partitions.
        bp_rep = const.tile([128, c], F32, tag="bp")
        nc.sync.dma_start(
            out=bp_rep,
            in_=bass.AP(tensor=bp_d[:].tensor, offset=0, ap=[[0, 128], [1, c]]),
        )

        # Persistent activations. Key order (j) is REVERSED within each
        # 128-chunk throughout the attention path (kT tiles, xTr -> vjones)
        # so the exp(bias) Toeplitz tiles become all-positive-stride Hankel
        # reads (walrus rejects negative outer strides in DMAs).
        acts = ctx.enter_context(tc.tile_pool(name="acts", bufs=1))
        xT = acts.tile([128, cb, n], F16, tag="xT")
        xTr = acts.tile([128, cb, n], F16, tag="xTr")
        qkT = [acts.tile([128, n], F16, tag=f"qkT{i}", name=f"qkT{i}") for i in range(qk_tiles)]
        vjones = [acts.tile([128, h, 2 * hd], F16, tag=f"vj{i}", name=f"vj{i}") for i in range(nb)]
        aoT = [acts.tile([128, n], F16, tag=f"aoT{i}", name=f"aoT{i}") for i in range(cb)]

        def rev_inner(ap_src, outer_pairs):
            """View with the innermost 128-wide free dim reversed."""
            return bass.AP(
                tensor=ap_src.tensor,
                offset=ap_src.offset + 127,
                ap=[ap_src.ap[0]] + outer_pairs + [[-1, 128]],
            )

        # ---- phase 1: x + noise -> fp16 -> xT via PE transpose
        with tc.tile_pool(name="ph1", bufs=3) as p1, \
             tc.tile_pool(name="ph1p", bufs=2, space="PSUM") as p1p:
            for a in range(nb):
                x32 = p1.tile([128, c], F32, tag="x32")
                nc.sync.dma_start(out=x32, in_=x_d[a * 128:(a + 1) * 128, :])
                xp = p1.tile([128, c], F16, tag="xp")
                nc.scalar.activation(
                    xp, x32, mybir.ActivationFunctionType.Identity,
                    bias=noise_sc[:, a:a + 1], scale=1.0,
                )
                ptp = p1p.tile([128, cb, 128], F16)
                for cc in range(cb):
                    nc.tensor.transpose(
                        out=ptp[:, cc, :], in_=xp[:, cc * 128:(cc + 1) * 128],
                        identity=ident,
                    )
                nc.scalar.copy(xT[:, :, a * 128:(a + 1) * 128], ptp[:])
                nc.scalar.copy(
                    xTr[:, :, a * 128:(a + 1) * 128],
                    rev_inner(ptp[:], [[128, cb]]),
                )

        # ---- phase 2: qT / kT = w_qk_block.T @ xT
        with tc.tile_pool(name="ph2w", bufs=6) as p2w, \
             tc.tile_pool(name="ph2p", bufs=2, space="PSUM") as p2p:
            for colb in range(qk_tiles):
                ps = p2p.tile([128, n], F32)
                for cc in range(cb):
                    wblk = p2w.tile([128, 128], F16)
                    nc.sync.dma_start(out=wblk, in_=wqk_d[cc, colb, :, :])
                    for j0, jl in n512:
                        nc.tensor.matmul(
                            ps[:, j0:j0 + jl], wblk, xT[:, cc, j0:j0 + jl],
                            start=(cc == 0), stop=(cc == cb - 1),
                        )
                if colb < cb:
                    nc.scalar.copy(qkT[colb], ps)
                else:
                    # k tiles: reverse key order within each 128-chunk
                    nc.scalar.copy(qkT[colb], rev_inner(ps[:], [[128, nb]]))

        # ---- phase 3: v = xT_block.T @ w_v, packed as [v_h | ones]
        with tc.tile_pool(name="ph3w", bufs=1) as p3w, \
             tc.tile_pool(name="ph3p", bufs=2, space="PSUM") as p3p:
            wv_sb = [p3w.tile([128, c], F16, tag=f"wv{cc}", name=f"wv{cc}") for cc in range(cb)]
            for cc in range(cb):
                nc.sync.dma_start(out=wv_sb[cc], in_=wv_d[cc * 128:(cc + 1) * 128, :])
            for a in range(nb):
                ps = p3p.tile([128, c], F32)
                for cc in range(cb):
                    for j0, jl in c512:
                        nc.tensor.matmul(
                            ps[:, j0:j0 + jl],
                            xTr[:, cc, a * 128:(a + 1) * 128],
                            wv_sb[cc][:, j0:j0 + jl],
                            start=(cc == 0), stop=(cc == cb - 1),
                        )
                nc.scalar.copy(
                    vjones[a][:, :, 0:hd],
                    ps.rearrange("p (hh d) -> p hh d", hh=h),
                )
                nc.vector.memset(vjones[a][:, :, hd:2 * hd], 1.0)

        # ---- phase 4: attention per (head, key-block)
        with tc.tile_pool(name="ph4e", bufs=6) as p4e, \
             tc.tile_pool(name="ph4x", bufs=4) as p4x, \
             tc.tile_pool(name="ph4a", bufs=4) as p4a, \
             tc.tile_pool(name="ph4f", bufs=2) as p4f, \
             tc.tile_pool(name="ph4ps", bufs=3, space="PSUM") as p4ps, \
             tc.tile_pool(name="ph4po", bufs=1, space="PSUM") as p4po:
            for hh in range(h):
                qt_i, qt_o = (hh * hd) // 128, (hh * hd) % 128
                qT_ap = qkT[qt_i][qt_o:qt_o + hd, :]
                kT_ap = qkT[cb + qt_i][qt_o:qt_o + hd, :]
                po = p4po.tile([128, n], F32)
                for jb in range(nb):
                    ps = p4ps.tile([128, n], F32)
                    for j0, jl in n512:
                        nc.tensor.matmul(
                            ps[:, j0:j0 + jl],
                            kT_ap[:, jb * 128:(jb + 1) * 128],
                            qT_ap[:, j0:j0 + jl],
                            start=True, stop=True,
                        )
                    et = p4e.tile([128, n], F16)
                    # row r holds key j = jb*128 + (127 - r); bias value is
                    # ebt[h, p - j + ws - 1] = ebt[h, (ws-128-128*jb) + r + p]
                    a0 = ws - 128 - 128 * jb
                    nc.sync.dma_start(
                        out=et,
                        in_=bass.AP(
                            tensor=ebt_ap.tensor,
                            offset=ebt_ap.offset + hh * tbl_len + a0,
                            ap=[[1, 128], [1, n]],
                        ),
                    )
                    ex = p4x.tile([128, n], F16)
                    nc.scalar.activation(
                        ex, ps, mybir.ActivationFunctionType.Exp, scale=scale,
                    )
                    at = p4a.tile([128, n], F16)
                    nc.vector.tensor_tensor(at, ex, et, op=mybir.AluOpType.mult)
                    for j0, jl in n512:
                        nc.tensor.matmul(
                            po[:, j0:j0 + jl],
                            vjones[jb][:, hh, :],
                            at[:, j0:j0 + jl],
                            start=(jb == 0), stop=(jb == nb - 1),
                        )
                # 1/rowsum as exp(-ln(rowsum)) on ACT: plain DVE reciprocal
                # costs ~6.5us per head here, the two table ops ~0.8us.
                lnr = p4f.tile([64, n], F32, tag="lnr")
                nc.scalar.activation(lnr, po[64:128, :], mybir.ActivationFunctionType.Ln)
                rc = p4f.tile([64, n], F32, tag="rc")
                nc.scalar.activation(rc, lnr, mybir.ActivationFunctionType.Exp, scale=-1.0)
                nc.vector.tensor_tensor(
                    aoT[qt_i][qt_o:qt_o + hd, :], po[0:hd, :], rc,
                    op=mybir.AluOpType.mult,
                )

        # ---- phase 5: out = attnout^T.T @ w_proj + b_proj
        with tc.tile_pool(name="ph5w", bufs=1) as p5w, \
             tc.tile_pool(name="ph5o", bufs=3) as p5o, \
             tc.tile_pool(name="ph5p", bufs=2, space="PSUM") as p5p:
            wp_sb = [p5w.tile([128, c], F16, tag=f"wp{cc}", name=f"wp{cc}") for cc in range(cb)]
            for cc in range(cb):
                nc.sync.dma_start(out=wp_sb[cc], in_=wp_d[cc * 128:(cc + 1) * 128, :])
            for a in range(nb):
                ps = p5p.tile([128, c], F32)
                for cc in range(cb):
                    for j0, jl in c512:
                        nc.tensor.matmul(
                            ps[:, j0:j0 + jl],
                            aoT[cc][:, a * 128:(a + 1) * 128],
                            wp_sb[cc][:, j0:j0 + jl],
                            start=(cc == 0), stop=(cc == cb - 1),
                        )
                ob = p5o.tile([128, c], F32)
                nc.vector.tensor_tensor(ob, ps, bp_rep[:, 0:c], op=mybir.AluOpType.add)
                nc.sync.dma_start(out=out_d[a * 128:(a + 1) * 128, :], in_=ob)

    return _split_multi_waits(nc)


def prep_core_inputs(x2d, noise2d, w_qkv, w_proj, b_proj, tbl, nstr, c=C):
    """Host-side input prep for one core: fp16 weight casts + blocking."""
    cb = c // 128
    qk_tiles = 2 * cb
    wqk = np.ascontiguousarray(
        w_qkv[:, : 2 * c].astype(np.float16)
        .reshape(cb, 128, qk_tiles, 128)
        .transpose(0, 2, 1, 3)
    )
    return dict(
        x=np.ascontiguousarray(x2d, dtype=np.float32),
        noise=np.ascontiguousarray(noise2d, dtype=np.float32),
        nstr=np.asarray(nstr, dtype=np.float32).reshape(1, 1),
        wqk=wqk,
        wv=np.ascontiguousarray(w_qkv[:, 2 * c:].astype(np.float16)),
        wproj=np.ascontiguousarray(w_proj.astype(np.float16)),
        bproj=np.ascontiguousarray(b_proj, dtype=np.float32),
        tbl=np.ascontiguousarray(tbl, dtype=np.float32),
    )


_NC_CACHE = {}


def get_nc():
    if "nc" not in _NC_CACHE:
        _NC_CACHE["nc"] = build()
    return _NC_CACHE["nc"]


def kernel(**inputs):
    x = np.asarray(inputs["x"], dtype=np.float32)
    noise = np.asarray(inputs["noise"], dtype=np.float32)
    w_qkv = np.asarray(inputs["w_qkv"], dtype=np.float32)
    w_proj = np.asarray(inputs["w_proj"], dtype=np.float32)
    b_proj = np.asarray(inputs["b_proj"], dtype=np.float32)
    tbl = np.asarray(inputs["rel_bias_table"], dtype=np.float32)
    nstr = np.asarray(inputs["noise_strength"], dtype=np.float32)

    shared = None
    in_maps = []
    for i in range(B):
        m = prep_core_inputs(x[i], noise[i], w_qkv, w_proj, b_proj, tbl, nstr)
        if shared is None:
            shared = {k: v for k, v in m.items() if k not in ("x", "noise")}
        else:
            for k in shared:
                m[k] = shared[k]
        in_maps.append(m)

    res = run_bass_kernel_spmd(get_nc(), in_maps, list(range(NCORES))).results
    return np.stack([res[i]["out"] for i in range(B)], axis=0).astype(np.float32)


if __name__ == "__main__":
    nc = build()
    print("build ok")


# revision 16
# speedup vs baseline: 1.1669x; 1.0180x over previous
"""Trainium2 Bass kernel for nn_Attention_84327387890534.

Multi-head attention with 1D relative position bias:
  x = x + noise * noise_strength
  qkv = x @ w_qkv -> q,k,v per head
  attn = softmax(q k^T * hd^-0.5 + rel_bias[i-j])
  out = (attn @ v) @ w_proj + b_proj

Sharding: data-parallel over batch B=8, one batch per NeuronCore.

Per-core design (all matmuls fp16 operands, fp32 PSUM accumulation):
  - x loaded fp32, noise-add fused into the fp16 cast (ACT per-partition
    bias), then PE-transposed to xT [c, n].
  - q,k computed transposed (qT/kT = [head*hd + d, n]) so scores need no
    per-head transposes; v computed natural [n, c] and packed per head as
    [v_h | ones] so the attn@v matmul also emits softmax row-sums for free
    (out rows 0..63 = unnormalized out^T, rows 64..127 = replicated rowsum).
  - scores computed transposed: S^T[j, p] tiles, softmax along free dim.
    Bias applied multiplicatively: exp(S + bias) = exp(S) * exp(bias); the
    exp(bias) Toeplitz tiles are generated by sliding-window DMAs (negative
    outer stride) from a per-head 2047-entry exp'd table staged in DRAM.
  - attn^T tiles feed the attn@v matmul directly (no transposes anywhere
    in the attention path); output is attnout^T [cin, n], which is exactly
    the lhsT layout the projection matmul needs.
"""

import sys

import numpy as np
from contextlib import ExitStack

try:
    import concourse.bass as bass
except ImportError:  # pragma: no cover
    sys.path.insert(0, "/opt/trn_rl_repo")
    import concourse.bass as bass

import concourse.tile as tile
from concourse import mybir
from concourse.bass_utils import run_bass_kernel_spmd
from concourse.masks import make_identity

F32 = mybir.dt.float32
F16 = mybir.dt.float16

# --- workaround: this walrus build rejects >1 sync-wait command on a single
# TPB_CTRL (Drain) instruction; TileContext's tail drain attaches every
# outstanding semaphore wait to one drain. Split the waits across extra
# drain instructions (2 per instruction) before the all-engine barrier.
_MAX_WAITS_PER_CTRL = 1


def _split_drain_and_barrier(self, tick_clock, wait_clock):
    import bass_rust
    from concourse.vector_clock import ScopedClock

    nc = self.nc
    drain_inst = nc.sync.drain()
    wait_clock.add_sem_waits(
        drain_inst.ins, ScopedClock({None: tick_clock.global_clock})
    )
    mi = drain_inst.ins
    si = mi.sync_info
    if si is not None and si.on_wait and len(si.on_wait) > _MAX_WAITS_PER_CTRL:
        waits = list(si.on_wait)
        mi.sync_info = bass_rust.SyncInfo(
            on_wait=waits[:_MAX_WAITS_PER_CTRL], on_update=list(si.on_update)
        )
        for i in range(_MAX_WAITS_PER_CTRL, len(waits), _MAX_WAITS_PER_CTRL):
            extra = nc.sync.drain()
            extra.ins.sync_info = bass_rust.SyncInfo(
                on_wait=waits[i:i + _MAX_WAITS_PER_CTRL], on_update=[]
            )

    nc.all_engine_barrier()
    assert self.sems is not None
    popped = nc._tile_sem_poison_stack.pop()
    assert popped is self._sem_poison
    nc.clear_and_free_semaphores(list(self.sems.allocated().values()))
    nc.all_engine_barrier()


tile.TileContext._drain_and_barrier = _split_drain_and_barrier


def _split_multi_waits(nc, max_waits=_MAX_WAITS_PER_CTRL):
    """Walrus here emits at most one sync-wait command per TPB instruction.
    Move excess semaphore waits onto same-engine NoOps inserted just before
    the over-subscribed instruction (identical semantics: engine streams
    are sequential, so the waits still all complete first)."""
    import bass_rust

    for fn in nc.m.functions:
        for bb in fn.blocks:
            out = []
            changed = False
            for inst in bb.instructions:
                si = inst.sync_info
                if si is not None and si.on_wait and len(si.on_wait) > max_waits:
                    waits = list(si.on_wait)
                    extras, keep = waits[:-max_waits], waits[-max_waits:]
                    for i in range(0, len(extras), max_waits):
                        nop = mybir.InstNoOp(
                            name=nc.get_next_instruction_name(), ins=[], outs=[]
                        )
                        nop.engine = inst.engine
                        nop.sync_info = bass_rust.SyncInfo(
                            on_wait=extras[i:i + max_waits], on_update=[]
                        )
                        nc.register_instruction(nop, overwrite=True)
                        out.append(nop)
                    inst.sync_info = bass_rust.SyncInfo(
                        on_wait=keep, on_update=list(si.on_update)
                    )
                    changed = True
                out.append(inst)
            if changed:
                bb.instructions = out
    return nc

# Problem dimensions (hardcoded per harness contract).
B = 8
N = 1024
C = 1024
H = 16
HD = 64
NCORES = 8


def build(n=N, c=C, h=H, hd=HD):
    """Build the single-core SPMD Bass program."""
    assert hd == 64 and c == h * hd and n % 128 == 0 and c % 128 == 0
    ws = n
    tbl_len = 2 * ws - 1
    nb, cb = n // 128, c // 128
    qk_tiles = 2 * cb
    scale = float(hd) ** -0.5
    n512 = [(j0, min(512, n - j0)) for j0 in range(0, n, 512)]
    c512 = [(j0, min(512, c - j0)) for j0 in range(0, c, 512)]

    nc = bass.Bass(trn_type="TRN2")
    x_d = nc.declare_dram_parameter("x", [n, c], F32, isOutput=False)
    nz_d = nc.declare_dram_parameter("noise", [n, 1], F32, isOutput=False)
    ns_d = nc.declare_dram_parameter("nstr", [1, 1], F32, isOutput=False)
    wqk_d = nc.declare_dram_parameter("wqk", [cb, qk_tiles, 128, 128], F16, isOutput=False)
    wv_d = nc.declare_dram_parameter("wv", [c, c], F16, isOutput=False)
    wp_d = nc.declare_dram_parameter("wproj", [c, c], F16, isOutput=False)
    bp_d = nc.declare_dram_parameter("bproj", [c], F32, isOutput=False)
    tb_d = nc.declare_dram_parameter("tbl", [tbl_len, h], F32, isOutput=False)
    out_d = nc.declare_dram_parameter("out", [n, c], F32, isOutput=True)

    with ExitStack() as ctx:
        tc = ctx.enter_context(tile.TileContext(nc))
        const = ctx.enter_context(tc.tile_pool(name="const", bufs=1))
        dramp = ctx.enter_context(tc.tile_pool(name="dram", bufs=1, space="DRAM"))

        ident = const.tile([128, 128], F16, tag="ident")
        make_identity(nc, ident)

        # exp(bias) table, transposed to [h, tbl_len] fp16, staged to DRAM.
        tblT = const.tile([h, tbl_len], F32, tag="tblT")
        nc.sync.dma_start(
            out=tblT,
            in_=bass.AP(tensor=tb_d[:].tensor, offset=0, ap=[[1, h], [h, tbl_len]]),
        )
        ebt_sb = const.tile([h, tbl_len], F16, tag="ebt_sb")
        nc.scalar.activation(ebt_sb, tblT, mybir.ActivationFunctionType.Exp)
        ebt = dramp.tile([h, tbl_len], F16)
        nc.sync.dma_start(out=ebt[:], in_=ebt_sb[:])
        ebt_ap = ebt[:]

        # noise * noise_strength: column a holds the bias for n-block a.
        nstr = const.tile([128, 1], F32, tag="nstr")
        nc.sync.dma_start(
            out=nstr,
            in_=bass.AP(tensor=ns_d[:].tensor, offset=0, ap=[[0, 128], [1, 1]]),
        )
        noise_sb = const.tile([128, nb], F32, tag="noise")
        nc.sync.dma_start(
            out=noise_sb,
            in_=bass.AP(tensor=nz_d[:].tensor, offset=0, ap=[[1, 128], [128, nb]]),
        )
        noise_sc = const.tile([128, nb], F32, tag="noise_sc")
        nc.vector.tensor_scalar_mul(noise_sc, noise_sb, nstr)

        # b_proj broadcast to all partitions.
        bp_rep = const.tile([128, c], F32, tag="bp")
        nc.sync.dma_start(
            out=bp_rep,
            in_=bass.AP(tensor=bp_d[:].tensor, offset=0, ap=[[0, 128], [1, c]]),
        )

        # Persistent activations. Key order (j) is REVERSED within each
        # 128-chunk throughout the attention path (kT tiles, xTr -> vjones)
        # so the exp(bias) Toeplitz tiles become all-positive-stride Hankel
        # reads (walrus rejects negative outer strides in DMAs).
        acts = ctx.enter_context(tc.tile_pool(name="acts", bufs=1))
        xT = acts.tile([128, cb, n], F16, tag="xT")
        xTr = acts.tile([128, cb, n], F16, tag="xTr")
        qkT = [acts.tile([128, n], F16, tag=f"qkT{i}", name=f"qkT{i}") for i in range(qk_tiles)]
        vjones = [acts.tile([128, h, 2 * hd], F16, tag=f"vj{i}", name=f"vj{i}") for i in range(nb)]
        aoT = [acts.tile([128, n], F16, tag=f"aoT{i}", name=f"aoT{i}") for i in range(cb)]

        def rev_inner(ap_src, outer_pairs):
            """View with the innermost 128-wide free dim reversed."""
            return bass.AP(
                tensor=ap_src.tensor,
                offset=ap_src.offset + 127,
                ap=[ap_src.ap[0]] + outer_pairs + [[-1, 128]],
            )

        # ---- phase 1: x + noise -> fp16 -> xT via PE transpose
        with tc.tile_pool(name="ph1", bufs=3) as p1, \
             tc.tile_pool(name="ph1p", bufs=2, space="PSUM") as p1p:
            for a in range(nb):
                x32 = p1.tile([128, c], F32, tag="x32")
                nc.sync.dma_start(out=x32, in_=x_d[a * 128:(a + 1) * 128, :])
                xp = p1.tile([128, c], F16, tag="xp")
                nc.scalar.activation(
                    xp, x32, mybir.ActivationFunctionType.Identity,
                    bias=noise_sc[:, a:a + 1], scale=1.0,
                )
                ptp = p1p.tile([128, cb, 128], F16)
                for cc in range(cb):
                    nc.tensor.transpose(
                        out=ptp[:, cc, :], in_=xp[:, cc * 128:(cc + 1) * 128],
                        identity=ident,
                    )
                nc.scalar.copy(xT[:, :, a * 128:(a + 1) * 128], ptp[:])
                nc.scalar.copy(
                    xTr[:, :, a * 128:(a + 1) * 128],
                    rev_inner(ptp[:], [[128, cb]]),
                )

        # ---- phase 2: qT / kT = w_qk_block.T @ xT
        with tc.tile_pool(name="ph2w", bufs=6) as p2w, \
             tc.tile_pool(name="ph2p", bufs=2, space="PSUM") as p2p:
            for colb in range(qk_tiles):
                ps = p2p.tile([128, n], F32)
                for cc in range(cb):
                    wblk = p2w.tile([128, 128], F16)
                    nc.sync.dma_start(out=wblk, in_=wqk_d[cc, colb, :, :])
                    for j0, jl in n512:
                        nc.tensor.matmul(
                            ps[:, j0:j0 + jl], wblk, xT[:, cc, j0:j0 + jl],
                            start=(cc == 0), stop=(cc == cb - 1),
                        )
                if colb < cb:
                    nc.vector.tensor_copy(qkT[colb], ps)
                else:
                    # k tiles: reverse key order within each 128-chunk
                    nc.scalar.copy(qkT[colb], rev_inner(ps[:], [[128, nb]]))

        # ---- phase 3: v = xT_block.T @ w_v, packed as [v_h | ones]
        with tc.tile_pool(name="ph3w", bufs=1) as p3w, \
             tc.tile_pool(name="ph3p", bufs=2, space="PSUM") as p3p:
            wv_sb = [p3w.tile([128, c], F16, tag=f"wv{cc}", name=f"wv{cc}") for cc in range(cb)]
            for cc in range(cb):
                nc.sync.dma_start(out=wv_sb[cc], in_=wv_d[cc * 128:(cc + 1) * 128, :])
            for a in range(nb):
                ps = p3p.tile([128, c], F32)
                for cc in range(cb):
                    for j0, jl in c512:
                        nc.tensor.matmul(
                            ps[:, j0:j0 + jl],
                            xTr[:, cc, a * 128:(a + 1) * 128],
                            wv_sb[cc][:, j0:j0 + jl],
                            start=(cc == 0), stop=(cc == cb - 1),
                        )
                nc.vector.tensor_copy(
                    vjones[a][:, :, 0:hd],
                    ps.rearrange("p (hh d) -> p hh d", hh=h),
                )
                nc.vector.memset(vjones[a][:, :, hd:2 * hd], 1.0)

        # ---- phase 4: attention per (head, key-block)
        # Heads run in interleaved pairs: two independent score->exp->mult->
        # accumulate chains keep the PE busy across each chain's ACT/DVE
        # latency (a single chain starves the PE and drops the HAM clock).
        with tc.tile_pool(name="ph4e", bufs=6) as p4e, \
             tc.tile_pool(name="ph4x", bufs=4) as p4x, \
             tc.tile_pool(name="ph4a", bufs=4) as p4a, \
             tc.tile_pool(name="ph4f", bufs=4) as p4f, \
             tc.tile_pool(name="ph4ps", bufs=2, space="PSUM") as p4ps, \
             tc.tile_pool(name="ph4po", bufs=2, space="PSUM") as p4po:

            def head_step(hh, jb, po):
                qt_i, qt_o = (hh * hd) // 128, (hh * hd) % 128
                qT_ap = qkT[qt_i][qt_o:qt_o + hd, :]
                kT_ap = qkT[cb + qt_i][qt_o:qt_o + hd, :]
                ps = p4ps.tile([128, n], F32, name="ps", tag="ps")
                for j0, jl in n512:
                    nc.tensor.matmul(
                        ps[:, j0:j0 + jl],
                        kT_ap[:, jb * 128:(jb + 1) * 128],
                        qT_ap[:, j0:j0 + jl],
                        start=True, stop=True,
                    )
                et = p4e.tile([128, n], F16, name="et", tag="et")
                # row r holds key j = jb*128 + (127 - r); bias value is
                # ebt[h, p - j + ws - 1] = ebt[h, (ws-128-128*jb) + r + p]
                a0 = ws - 128 - 128 * jb
                nc.sync.dma_start(
                    out=et,
                    in_=bass.AP(
                        tensor=ebt_ap.tensor,
                        offset=ebt_ap.offset + hh * tbl_len + a0,
                        ap=[[1, 128], [1, n]],
                    ),
                )
                ex = p4x.tile([128, n], F16, name="ex", tag="ex")
                nc.scalar.activation(
                    ex, ps, mybir.ActivationFunctionType.Exp, scale=scale,
                )
                at = p4a.tile([128, n], F16, name="at", tag="at")
                nc.vector.tensor_tensor(at, ex, et, op=mybir.AluOpType.mult)
                for j0, jl in n512:
                    nc.tensor.matmul(
                        po[:, j0:j0 + jl],
                        vjones[jb][:, hh, :],
                        at[:, j0:j0 + jl],
                        start=(jb == 0), stop=(jb == nb - 1),
                    )

            def head_fin(hh, po):
                qt_i, qt_o = (hh * hd) // 128, (hh * hd) % 128
                # 1/rowsum as exp(-ln(rowsum)) on ACT: plain DVE reciprocal
                # costs ~6.5us per head here, the two table ops ~0.8us.
                lnr = p4f.tile([64, n], F32, name="lnr", tag="lnr")
                nc.scalar.activation(
                    lnr, po[64:128, :], mybir.ActivationFunctionType.Ln)
                rc = p4f.tile([64, n], F32, name="rc", tag="rc")
                nc.scalar.activation(
                    rc, lnr, mybir.ActivationFunctionType.Exp, scale=-1.0)
                nc.vector.tensor_tensor(
                    aoT[qt_i][qt_o:qt_o + hd, :], po[0:hd, :], rc,
                    op=mybir.AluOpType.mult,
                )

            for g in range(h // 2):
                hA, hB = 2 * g, 2 * g + 1
                poA = p4po.tile([128, n], F32, name="poA", tag="po")
                poB = p4po.tile([128, n], F32, name="poB", tag="po")
                for jb in range(nb):
                    head_step(hA, jb, poA)
                    head_step(hB, jb, poB)
                head_fin(hA, poA)
                head_fin(hB, poB)

        # ---- phase 5: out = attnout^T.T @ w_proj + b_proj
        with tc.tile_pool(name="ph5w", bufs=1) as p5w, \
             tc.tile_pool(name="ph5o", bufs=3) as p5o, \
             tc.tile_pool(name="ph5p", bufs=2, space="PSUM") as p5p:
            wp_sb = [p5w.tile([128, c], F16, tag=f"wp{cc}", name=f"wp{cc}") for cc in range(cb)]
            for cc in range(cb):
                nc.sync.dma_start(out=wp_sb[cc], in_=wp_d[cc * 128:(cc + 1) * 128, :])
            for a in range(nb):
                ps = p5p.tile([128, c], F32)
                for cc in range(cb):
                    for j0, jl in c512:
                        nc.tensor.matmul(
                            ps[:, j0:j0 + jl],
                            aoT[cc][:, a * 128:(a + 1) * 128],
                            wp_sb[cc][:, j0:j0 + jl],
                            start=(cc == 0), stop=(cc == cb - 1),
                        )
                ob = p5o.tile([128, c], F32)
                nc.vector.tensor_tensor(ob, ps, bp_rep[:, 0:c], op=mybir.AluOpType.add)
                nc.sync.dma_start(out=out_d[a * 128:(a + 1) * 128, :], in_=ob)

    return _split_multi_waits(nc)


def prep_core_inputs(x2d, noise2d, w_qkv, w_proj, b_proj, tbl, nstr, c=C):
    """Host-side input prep for one core: fp16 weight casts + blocking."""
    cb = c // 128
    qk_tiles = 2 * cb
    wqk = np.ascontiguousarray(
        w_qkv[:, : 2 * c].astype(np.float16)
        .reshape(cb, 128, qk_tiles, 128)
        .transpose(0, 2, 1, 3)
    )
    return dict(
        x=np.ascontiguousarray(x2d, dtype=np.float32),
        noise=np.ascontiguousarray(noise2d, dtype=np.float32),
        nstr=np.asarray(nstr, dtype=np.float32).reshape(1, 1),
        wqk=wqk,
        wv=np.ascontiguousarray(w_qkv[:, 2 * c:].astype(np.float16)),
        wproj=np.ascontiguousarray(w_proj.astype(np.float16)),
        bproj=np.ascontiguousarray(b_proj, dtype=np.float32),
        tbl=np.ascontiguousarray(tbl, dtype=np.float32),
    )


_NC_CACHE = {}


def get_nc():
    if "nc" not in _NC_CACHE:
        _NC_CACHE["nc"] = build()
    return _NC_CACHE["nc"]


def kernel(**inputs):
    x = np.asarray(inputs["x"], dtype=np.float32)
    noise = np.asarray(inputs["noise"], dtype=np.float32)
    w_qkv = np.asarray(inputs["w_qkv"], dtype=np.float32)
    w_proj = np.asarray(inputs["w_proj"], dtype=np.float32)
    b_proj = np.asarray(inputs["b_proj"], dtype=np.float32)
    tbl = np.asarray(inputs["rel_bias_table"], dtype=np.float32)
    nstr = np.asarray(inputs["noise_strength"], dtype=np.float32)

    shared = None
    in_maps = []
    for i in range(B):
        m = prep_core_inputs(x[i], noise[i], w_qkv, w_proj, b_proj, tbl, nstr)
        if shared is None:
            shared = {k: v for k, v in m.items() if k not in ("x", "noise")}
        else:
            for k in shared:
                m[k] = shared[k]
        in_maps.append(m)

    res = run_bass_kernel_spmd(get_nc(), in_maps, list(range(NCORES))).results
    return np.stack([res[i]["out"] for i in range(B)], axis=0).astype(np.float32)


if __name__ == "__main__":
    nc = build()
    print("build ok")
